# revision 41
# baseline (speedup 1.0000x reference)
"""Causal self-attention (B=2, T=2048, C=1024, H=16) on 8 TRN2 NeuronCores.

Sharding: (batch x head-group). Core (b, g) owns batch b and 4 heads
(2 head-pairs). It computes q/k/v projections for its 256 feature columns
over its batch's 2048 tokens, full causal attention for its (b, head) pairs,
and a partial output projection against its 256 rows of w_proj. The 4
partial [2048, 1024] outputs per batch are summed on host and b_proj is
added once during that reduction.

Within a core everything is "transposed" (features on partitions, tokens on
the free dim): xT [C, Tb] --PE--> Q^T/K^T/V^T planes [128, 2hp, 2048] and
V^T --PE transpose--> V (token-major, with an interleaved ones column per
head so softmax denominators fall out of the AV matmul).

All matmul operands are bf16: the PE streams 1 column/cycle regardless of
dtype but fp32 runs at half rate (fp32_mode=HIGH), so bf16 doubles matmul
throughput and enables fast weight loads. fp32 accumulation in PSUM
throughout; softmax denominators and reciprocals stay fp32. Softmax skips
the max-subtraction: scores are ~N(0,1) (bounded ~+-6), far inside exp
range.

Causal structure: per 512-wide q-chunk, diagonal k-tiles restrict the S
matmul / exp / AV matmul to the causally needed q-columns, and the mask
multiply (on the otherwise-idle GpSimd engine) touches only the 128-wide
crossing block.

Emission is software-pipelined: S matmuls run one k-tile ahead of the AV
matmuls, and independent filler work — the next token-slab's QKV projection
and V-transposes, plus the previous chunk's deferred output projection — is
drip-fed between S(i+1) and AV(i) at single-matmul granularity. This keeps
the in-order PE queue continuously busy while ScalarE (the only exp-capable
engine, ~1.1us per k-tile) paces the attention inner loop: PE idle gaps both
reset the P-state ramp (2.4 GHz only after ~3us of continuous execution) and
trip the HAM power manager down to half the array. Each slab's hp1 units are
carried into the chunk that first reads them, and the last chunk's output
projection is split per head-pair plane (the hp1 half lands in a separate
`y2` partial summed on host) so the kernel tail stays short.
"""

import numpy as np
import ml_dtypes

import concourse.bass as bass
import concourse.mybir as mybir
import concourse.tile as tile
from concourse import bacc
from concourse.bass_utils import run_bass_kernel_spmd
from concourse.masks import make_identity

F32 = mybir.dt.float32
F32R = mybir.dt.float32r
BF16 = mybir.dt.bfloat16
EXP = mybir.ActivationFunctionType.Exp

B, T, C = 2, 2048, 1024
H, DH = 16, 64
NCORES = 8
NG = 4                    # head-groups
FPC = 256                 # q/k/v feature columns per core (4 heads)
NKT = T // 128            # 16 k-tiles per batch
NSLAB = T // 512          # 4 token slabs / q-chunks
SCALE = DH ** -0.5

_CACHE = {}


def _build():
    nc = bacc.Bacc(
        "TRN2",
        target_bir_lowering=False,
        debug=False,
        enable_asserts=True,
        num_devices=NCORES,
    )
    xT = nc.dram_tensor("xT", [C, T], BF16, kind="ExternalInput").ap()
    wq = nc.dram_tensor("wq", [C, FPC], BF16, kind="ExternalInput").ap()
    wk = nc.dram_tensor("wk", [C, FPC], BF16, kind="ExternalInput").ap()
    wv = nc.dram_tensor("wv", [C, FPC], BF16, kind="ExternalInput").ap()
    bq = nc.dram_tensor("bq", [128, 2], F32, kind="ExternalInput").ap()
    bk = nc.dram_tensor("bk", [128, 2], F32, kind="ExternalInput").ap()
    bv = nc.dram_tensor("bv", [128, 2], F32, kind="ExternalInput").ap()
    wp = nc.dram_tensor("wp", [FPC, C], BF16, kind="ExternalInput").ap()
    y = nc.dram_tensor("y", [T, C], BF16, kind="ExternalOutput").ap()
    # last chunk's hp1 output-projection partial (host adds it into y's last
    # 512 rows) — lets the hp0 half run as filler instead of serializing the
    # whole projection after the final softmax
    y2 = nc.dram_tensor("y2", [512, C], BF16, kind="ExternalOutput").ap()

    with tile.TileContext(nc) as tc:
        with (
            tc.tile_pool(name="const", bufs=1) as cst,
            tc.tile_pool(name="qkvt", bufs=1) as qkvt,
            tc.tile_pool(name="xin", bufs=2) as xin,
            tc.tile_pool(name="ptile", bufs=3) as ptile,
            tc.tile_pool(name="attn", bufs=4) as attnp,
            tc.tile_pool(name="yout", bufs=3) as yout,
            tc.tile_pool(name="small", bufs=2) as small,
            tc.tile_pool(name="psum", bufs=2, space="PSUM") as ps,
        ):
            # ---- constants ----
            ident = cst.tile([128, 128], BF16, tag="ident", name="ident")
            make_identity(nc, ident)

            # Crossing-block causal mask, multiplicative, replicated per head:
            # M2[p, h, j] = 1.0 iff j >= p.
            M2 = cst.tile([128, 2, 128], BF16, tag="mask", name="mask")
            nc.vector.memset(M2, 1.0)
            for h in range(2):
                nc.gpsimd.affine_select(
                    out=M2[:, h, :],
                    in_=M2[:, h, :],
                    compare_op=mybir.AluOpType.is_ge,
                    fill=0.0,
                    base=0,
                    pattern=[[1, 128]],
                    channel_multiplier=-1,
                )

            # ones row at partition 64 (bf16): stationary operand of the K=1
            # matmul that broadcasts the softmax denominator from partition 64
            # down to partitions 0..63.
            ones64r = cst.tile([128, 64], BF16, tag="ones64r", name="ones64r")
            nc.vector.memset(ones64r[64:65, :], 1.0)

            # ---- persistent activations ----
            QT = qkvt.tile([128, 2, T], BF16, tag="QT", name="QT")
            KT = qkvt.tile([128, 2, T], BF16, tag="KT", name="KT")
            VT = qkvt.tile([128, 2, T], BF16, tag="VT", name="VT")
            # V per hp-plane, token-major, per k-tile block of 130 cols:
            # [64 V_h0 | 1 | 64 V_h1 | 1]. memset 1.0 once; value columns are
            # overwritten by the transpose evacuations.
            V = qkvt.tile([128, 2, NKT, 130], BF16, tag="V", name="V")
            nc.vector.memset(V, 1.0)

            xts = [None] * NSLAB
            x_view = xT.rearrange("(ct p) t -> p ct t", p=128)

            def dma_slab(tj, split=False):
                xt = xin.tile([128, C // 128, 512], BF16, tag="xt", name="xt")
                if split:
                    # two halves so the first projection matmuls can start
                    # once their contraction chunks land (subtile deps), while
                    # keeping Sync-engine dispatch cost (~0.6us each) low
                    for h0 in (0, 4):
                        nc.sync.dma_start(
                            out=xt[:, h0 : h0 + 4, :],
                            in_=x_view[:, h0 : h0 + 4, tj * 512 : (tj + 1) * 512],
                        )
                else:
                    nc.sync.dma_start(
                        out=xt, in_=x_view[:, :, tj * 512 : (tj + 1) * 512]
                    )
                xts[tj] = xt

            # ---- weights / biases: wk + xt slab 0 first (they gate the
            # first projection); wp (only needed from chunk 1 on) last ----
            w_sb = {}
            b_sb = {}
            for name in ("k", "q", "v"):
                w_sb[name] = cst.tile(
                    [128, C // 128, FPC], BF16, tag=f"w{name}", name=f"w{name}"
                )
            wviews = {"k": wk, "q": wq, "v": wv}
            wk_view = wk.rearrange("(ct p) f -> p ct f", p=128)
            nc.sync.dma_start(out=w_sb["k"][:, 0:4, :], in_=wk_view[:, 0:4, :])
            dma_slab(0, split=True)
            nc.sync.dma_start(out=w_sb["k"][:, 4:8, :], in_=wk_view[:, 4:8, :])
            for name in ("q", "v"):
                nc.sync.dma_start(
                    out=w_sb[name],
                    in_=wviews[name].rearrange("(ct p) f -> p ct f", p=128),
                )
            for name, bap in (("k", bk), ("q", bq), ("v", bv)):
                b_sb[name] = cst.tile([128, 2], F32, tag=f"b{name}", name=f"b{name}")
                nc.sync.dma_start(out=b_sb[name], in_=bap)
            wp_sb = cst.tile([128, 2, C], BF16, tag="wp", name="wp")
            nc.sync.dma_start(
                out=wp_sb, in_=wp.rearrange("(hp p) f -> p hp f", p=128)
            )

            out_plane = {"q": QT, "k": KT, "v": VT}

            # ---- filler units: generators yielding once per PE instruction ----
            # The PE queue executes in order; to keep it continuously busy
            # (P-state ramps to 2.4 GHz only after ~3us without idle gaps)
            # filler work is drip-fed between attention k-tiles at single-
            # matmul granularity.

            def proj_unit(name, hp, tj):
                acc = ps.tile([128, 512], F32, tag="acc", name="acc")
                for ct in range(C // 128):
                    nc.tensor.matmul(
                        acc,
                        w_sb[name][:, ct, 128 * hp : 128 * hp + 128],
                        xts[tj][:, ct, :],
                        start=(ct == 0),
                        stop=(ct == C // 128 - 1),
                    )
                    yield
                nc.vector.tensor_scalar_add(
                    out_plane[name][:, hp, tj * 512 : (tj + 1) * 512],
                    acc,
                    b_sb[name][:, hp : hp + 1],
                )
                yield

            def transpose_unit(hp, tj):
                # 4 k-tiles' transposes batched into one PSUM tile + one copy
                pv4 = ps.tile([128, 4, 128], BF16, tag="acc", name="pv4")
                for j in range(4):
                    kt = 4 * tj + j
                    nc.tensor.transpose(
                        pv4[:, j, :], VT[:, hp, kt * 128 : (kt + 1) * 128], ident
                    )
                    yield
                nc.vector.tensor_copy(
                    out=V[:, hp, 4 * tj : 4 * tj + 4, :].rearrange(
                        "p k (s c) -> p k s c", c=65
                    )[:, :, :, 0:64],
                    in_=pv4.rearrange("p k (s c) -> p k s c", c=64),
                )
                yield

            def outproj_unit(qc, a0, a1t):
                q0 = qc * 512
                for tt in range(4):
                    yp = [
                        ps.tile([128, 512], F32, tag="acc", name="yp")
                        for _ in range(2)
                    ]
                    for cc in range(2):
                        nc.tensor.matmul(
                            yp[cc],
                            a0[:, tt * 128 : (tt + 1) * 128],
                            wp_sb[:, 0, cc * 512 : (cc + 1) * 512],
                            start=True,
                            stop=False,
                        )
                        yield
                    for cc in range(2):
                        nc.tensor.matmul(
                            yp[cc],
                            a1t[:, tt * 128 : (tt + 1) * 128],
                            wp_sb[:, 1, cc * 512 : (cc + 1) * 512],
                            start=False,
                            stop=True,
                        )
                        yield
                    ysb = yout.tile([128, C], BF16, tag="ysb", name="ysb")
                    nc.vector.tensor_copy(ysb[:, 0:512], yp[0])
                    nc.scalar.copy(ysb[:, 512:1024], yp[1])
                    t0 = q0 + tt * 128
                    nc.sync.dma_start(out=y[t0 : t0 + 128, :], in_=ysb)
                    yield

            def slab_units(tj, hps=(0, 1)):
                units = []
                for name in ("k", "q", "v"):
                    for hp in hps:
                        units.append((proj_unit(name, hp, tj), 9))
                for hp in hps:
                    units.append((transpose_unit(hp, tj), 5))
                return units

            # ---- per-chunk attention with software-pipelined S and filler ----
            def emit_s(hp, kt, qc):
                m = kt - 4 * qc
                w0 = 128 * m if m >= 0 else 0
                k0 = kt * 128
                q0 = qc * 512
                s = ps.tile([128, 2, 512], F32, tag="s", name="s")
                for h in range(2):
                    nc.tensor.matmul(
                        s[:, h, w0:512],
                        KT[64 * h : 64 * h + 64, hp, k0 : k0 + 128],
                        QT[64 * h : 64 * h + 64, hp, q0 + w0 : q0 + 512],
                        start=True,
                        stop=True,
                    )
                pt = ptile.tile([128, 2, 512], BF16, tag="pt", name="pt")
                nc.scalar.activation(
                    out=pt[:, :, w0:512], in_=s[:, :, w0:512], func=EXP, scale=SCALE
                )
                if m >= 0:
                    nc.gpsimd.tensor_mul(
                        pt[:, :, w0 : w0 + 128], pt[:, :, w0 : w0 + 128], M2
                    )
                return pt

            def outproj_half(plane, at, target, t0):
                # single-plane output projection partial (last chunk only)
                for tt in range(4):
                    yp = [
                        ps.tile([128, 512], F32, tag="acc", name="yph")
                        for _ in range(2)
                    ]
                    for cc in range(2):
                        nc.tensor.matmul(
                            yp[cc],
                            at[:, tt * 128 : (tt + 1) * 128],
                            wp_sb[:, plane, cc * 512 : (cc + 1) * 512],
                            start=True,
                            stop=True,
                        )
                        yield
                    ysb = yout.tile([128, C], BF16, tag="ysb", name="ysb")
                    nc.vector.tensor_copy(ysb[:, 0:512], yp[0])
                    nc.scalar.copy(ysb[:, 512:1024], yp[1])
                    r0 = t0 + tt * 128
                    nc.sync.dma_start(out=target[r0 : r0 + 128, :], in_=ysb)
                    yield

            def normalize(av, attn):
                for h in range(2):
                    denr = small.tile([128, 512], BF16, tag="denr", name="denr")
                    nc.vector.tensor_copy(out=denr[64:65, :], in_=av[h][64:65, :])
                    bc = ps.tile([64, 512], F32, tag="acc", name="bc")
                    nc.tensor.matmul(
                        bc, ones64r[64:65, :], denr[64:65, :], start=True, stop=True
                    )
                    rbc = small.tile([64, 512], F32, tag="rbc", name="rbc")
                    nc.vector.reciprocal_approx_fast(rbc, bc)
                    if h == 0:
                        nc.vector.tensor_mul(attn[0:64, :], av[h][0:64, :], rbc)
                    else:
                        a1 = attnp.tile([64, 512], BF16, tag="attn1", name="a1")
                        nc.vector.tensor_mul(a1, av[h][0:64, :], rbc)
                        # cross-partition move: only cheap path is DMA
                        nc.sync.dma_start(out=attn[64:128, :], in_=a1)

            # ---- prologue: slab 0 (hp0 eagerly; hp1 as chunk-0 filler) ----
            for gen, _ in slab_units(0, hps=(0,)):
                for _ in gen:
                    pass

            # Each slab's hp1 units are carried into the chunk that first
            # reads them, so every chunk — including the last, biggest one —
            # has filler to keep the PE busy while ScalarE drains exps.
            carry = slab_units(0, hps=(1,))
            op_carry = None
            guard_pre = 0
            carry2 = []
            for qc in range(NSLAB):
                carry_steps = sum(n for _, n in carry)
                pending = list(carry)
                # the deferred output projection is injected at the hp
                # boundary so it feeds the filler-starved hp1 phase
                late = [op_carry] if op_carry else []
                op_carry = None
                carry = []
                g_pre = guard_pre
                guard_pre = 0
                late2 = carry2
                carry2 = []
                if qc + 1 < NSLAB:
                    dma_slab(qc + 1)
                    if qc + 1 == NSLAB - 1:
                        # the last, biggest chunk needs the most filler, and
                        # its late hp1 phase needs it most. Units are carried
                        # to just before their first reader: k/v/transpose
                        # hp0 by k-tile 12 of the hp0 pass; q-hp1 by the hp
                        # boundary; k/v/transpose hp1 by k-tile 12 of the hp1
                        # pass. Only the hp0 q-projection (read by the
                        # chunk's first S) stays here.
                        pending += [(proj_unit("q", 0, qc + 1), 9)]
                        carry = [
                            (proj_unit("k", 0, qc + 1), 9),
                            (proj_unit("v", 0, qc + 1), 9),
                            (transpose_unit(0, qc + 1), 5),
                            (proj_unit("q", 1, qc + 1), 9),
                        ]
                        guard_pre = 23
                        carry2 = [
                            (proj_unit("k", 1, qc + 1), 9),
                            (proj_unit("v", 1, qc + 1), 9),
                            (transpose_unit(1, qc + 1), 5),
                        ]
                    else:
                        pending += slab_units(qc + 1, hps=(0,))
                        carry = slab_units(qc + 1, hps=(1,))
                total_steps = sum(n for _, n in pending)
                nkt = 4 * (qc + 1)
                iters = [(hp, kt) for hp in range(2) for kt in range(nkt)]
                steps_done = 0

                def advance(target):
                    nonlocal steps_done, pending
                    while pending and steps_done < target:
                        gen, n = pending[0]
                        try:
                            next(gen)
                            steps_done += 1
                        except StopIteration:
                            pending.pop(0)

                av = {}
                pts = {0: emit_s(*iters[0], qc)}
                attn_tiles = []
                for i, (hp, kt) in enumerate(iters):
                    if i + 1 < len(iters):
                        if g_pre and i + 1 == nkt - 4:
                            # carried hp0 k/v/transpose units (front of
                            # `pending`) must be emitted before the k-tile-12
                            # S/AV matmuls that read them
                            advance(g_pre)
                        if i + 1 == nkt:
                            # this chunk's hp1 plane is produced by the
                            # carried units at the front of `pending` — they
                            # must be emitted before any reader (deps follow
                            # emission order)
                            advance(carry_steps)
                            pending.extend(late)
                            total_steps += sum(n for _, n in late)
                            late = []
                        if late2 and i + 1 == nkt + 4 * qc:
                            # deep-hp1 filler: slab units whose first reader
                            # is the hp1 pass's k-tile 4*qc — force-emit them
                            # here (burst keeps the PE queue deep through the
                            # chunk tail)
                            pending.extend(late2)
                            total_steps += sum(n for _, n in late2)
                            late2 = []
                            advance(total_steps)
                        pts[i + 1] = emit_s(*iters[i + 1], qc)
                    # filler goes between S(i+1) and AV(i) in the in-order PE
                    # queue: the PE chews filler while ScalarE finishes exp(i)
                    advance((i + 1) * total_steps // len(iters))
                    m = kt - 4 * qc
                    w0 = 128 * m if m >= 0 else 0
                    if kt == 0:
                        av[hp] = [
                            ps.tile([65, 512], F32, tag="av", name="av")
                            for _ in range(2)
                        ]
                    pt = pts.pop(i)
                    for h in range(2):
                        nc.tensor.matmul(
                            av[hp][h][:, w0:512],
                            V[:, hp, kt, 65 * h : 65 * h + 65],
                            pt[:, h, w0:512],
                            start=(kt == 0),
                            stop=(kt == nkt - 1),
                        )
                    if kt == nkt - 1:
                        attn = attnp.tile([128, 512], BF16, tag="attn", name="attn")
                        normalize(av[hp], attn)
                        attn_tiles.append(attn)
                        if qc == NSLAB - 1 and hp == 0:
                            # last chunk: hp0's projection half feeds the PE
                            # during the hp1 phase. Its steps MUST be counted
                            # in total_steps: the late2 guard targets are
                            # step counts over the front-ordered queue, and
                            # an uncounted unit ahead of late2 would absorb
                            # the guard budget and leave late2 unemitted
                            # before its readers.
                            pending.append(
                                (outproj_half(0, attn, y, qc * 512), 12)
                            )
                            total_steps += 12
                        # keep the PE fed while the DVE normalize chain frees
                        # the av buffers the next head-pair's AVs rotate onto
                        advance(steps_done + 6)

                while pending:
                    advance(steps_done + 100)

                if qc + 1 < NSLAB:
                    op_carry = (outproj_unit(qc, *attn_tiles), 20)
                else:
                    for _ in outproj_half(1, attn_tiles[1], y2, 0):
                        pass

    nc.compile()
    return nc


def _get_nc():
    if "nc" not in _CACHE:
        _CACHE["nc"] = _build()
    return _CACHE["nc"]


def _bf16(a):
    return np.ascontiguousarray(a.astype(ml_dtypes.bfloat16))


def _run(inputs, **spmd_kwargs):
    x = np.asarray(inputs["x"], dtype=np.float32)
    w_qkv = np.asarray(inputs["w_qkv"], dtype=np.float32)
    b_qkv = np.asarray(inputs["b_qkv"], dtype=np.float32)
    w_proj = np.asarray(inputs["w_proj"], dtype=np.float32)
    b_proj = np.asarray(inputs["b_proj"], dtype=np.float32)

    nc = _get_nc()

    in_maps = []
    for i in range(NCORES):
        b = i // NG
        g = i % NG
        f0 = g * FPC
        sl = slice(f0, f0 + FPC)
        in_maps.append(
            {
                "xT": _bf16(x[b].T),
                "wq": _bf16(w_qkv[:, sl]),
                "wk": _bf16(w_qkv[:, C + f0 : C + f0 + FPC]),
                "wv": _bf16(w_qkv[:, 2 * C + f0 : 2 * C + f0 + FPC]),
                "bq": np.ascontiguousarray(
                    b_qkv[sl].reshape(2, 128).T
                ),
                "bk": np.ascontiguousarray(
                    b_qkv[C + f0 : C + f0 + FPC].reshape(2, 128).T
                ),
                "bv": np.ascontiguousarray(
                    b_qkv[2 * C + f0 : 2 * C + f0 + FPC].reshape(2, 128).T
                ),
                "wp": _bf16(w_proj[sl, :]),
            }
        )

    res = run_bass_kernel_spmd(nc, in_maps, core_ids=list(range(NCORES)), **spmd_kwargs)
    acc = np.zeros((B, T, C), dtype=np.float64)
    for i, om in enumerate(res.results):
        acc[i // NG] += np.asarray(om["y"]).astype(np.float64)
        # last chunk's hp1 output-projection partial
        acc[i // NG, T - 512 :] += np.asarray(om["y2"]).astype(np.float64)
    out = (acc + b_proj.astype(np.float64)).astype(np.float32)
    return out, res


def kernel(**inputs) -> np.ndarray:
    out, _ = _run(inputs)
    return out


# revision 42
# speedup vs baseline: 1.0132x; 1.0132x over previous
"""Causal self-attention (B=2, T=2048, C=1024, H=16) on 8 TRN2 NeuronCores.

Sharding: (batch x head-group). Core (b, g) owns batch b and 4 heads
(2 head-pairs). It computes q/k/v projections for its 256 feature columns
over its batch's 2048 tokens, full causal attention for its (b, head) pairs,
and a partial output projection against its 256 rows of w_proj. The 4
partial [2048, 1024] outputs per batch are summed on host and b_proj is
added once during that reduction.

Within a core everything is "transposed" (features on partitions, tokens on
the free dim): xT [C, Tb] --PE--> Q^T/K^T/V^T planes [128, 2hp, 2048] and
V^T --PE transpose--> V (token-major, with an interleaved ones column per
head so softmax denominators fall out of the AV matmul).

All matmul operands are bf16: the PE streams 1 column/cycle regardless of
dtype but fp32 runs at half rate (fp32_mode=HIGH), so bf16 doubles matmul
throughput and enables fast weight loads. fp32 accumulation in PSUM
throughout; softmax denominators and reciprocals stay fp32. Softmax skips
the max-subtraction: scores are ~N(0,1) (bounded ~+-6), far inside exp
range.

Causal structure: per 512-wide q-chunk, diagonal k-tiles restrict the S
matmul / exp / AV matmul to the causally needed q-columns, and the mask
multiply (on the otherwise-idle GpSimd engine) touches only the 128-wide
crossing block.

Emission is software-pipelined: S matmuls run one k-tile ahead of the AV
matmuls, and independent filler work — the next token-slab's QKV projection
and V-transposes, plus the previous chunk's deferred output projection — is
drip-fed between S(i+1) and AV(i) at single-matmul granularity. This keeps
the in-order PE queue continuously busy while ScalarE (the only exp-capable
engine, ~1.1us per k-tile) paces the attention inner loop: PE idle gaps both
reset the P-state ramp (2.4 GHz only after ~3us of continuous execution) and
trip the HAM power manager down to half the array. Each slab's hp1 units are
carried into the chunk that first reads them, and the last chunk's output
projection is split per head-pair plane (the hp1 half lands in a separate
`y2` partial summed on host) so the kernel tail stays short.
"""

import numpy as np
import ml_dtypes

import concourse.bass as bass
import concourse.mybir as mybir
import concourse.tile as tile
from concourse import bacc
from concourse.bass_utils import run_bass_kernel_spmd
from concourse.masks import make_identity

F32 = mybir.dt.float32
F32R = mybir.dt.float32r
BF16 = mybir.dt.bfloat16
EXP = mybir.ActivationFunctionType.Exp

B, T, C = 2, 2048, 1024
H, DH = 16, 64
NCORES = 8
NG = 4                    # head-groups
FPC = 256                 # q/k/v feature columns per core (4 heads)
NKT = T // 128            # 16 k-tiles per batch
NSLAB = T // 512          # 4 token slabs / q-chunks
SCALE = DH ** -0.5

_CACHE = {}


def _build():
    nc = bacc.Bacc(
        "TRN2",
        target_bir_lowering=False,
        debug=False,
        enable_asserts=True,
        num_devices=NCORES,
    )
    xT = nc.dram_tensor("xT", [C, T], BF16, kind="ExternalInput").ap()
    wq = nc.dram_tensor("wq", [C, FPC], BF16, kind="ExternalInput").ap()
    wk = nc.dram_tensor("wk", [C, FPC], BF16, kind="ExternalInput").ap()
    wv = nc.dram_tensor("wv", [C, FPC], BF16, kind="ExternalInput").ap()
    bq = nc.dram_tensor("bq", [128, 2], F32, kind="ExternalInput").ap()
    bk = nc.dram_tensor("bk", [128, 2], F32, kind="ExternalInput").ap()
    bv = nc.dram_tensor("bv", [128, 2], F32, kind="ExternalInput").ap()
    wp = nc.dram_tensor("wp", [FPC, C], BF16, kind="ExternalInput").ap()
    y = nc.dram_tensor("y", [T, C], BF16, kind="ExternalOutput").ap()
    # last chunk's hp1 output-projection partial (host adds it into y's last
    # 512 rows) — lets the hp0 half run as filler instead of serializing the
    # whole projection after the final softmax
    y2 = nc.dram_tensor("y2", [512, C], BF16, kind="ExternalOutput").ap()

    with tile.TileContext(nc) as tc:
        with (
            tc.tile_pool(name="const", bufs=1) as cst,
            tc.tile_pool(name="qkvt", bufs=1) as qkvt,
            tc.tile_pool(name="xin", bufs=2) as xin,
            tc.tile_pool(name="ptile", bufs=3) as ptile,
            tc.tile_pool(name="attn", bufs=4) as attnp,
            tc.tile_pool(name="yout", bufs=3) as yout,
            tc.tile_pool(name="small", bufs=2) as small,
            tc.tile_pool(name="psum", bufs=2, space="PSUM") as ps,
        ):
            # ---- constants ----
            ident = cst.tile([128, 128], BF16, tag="ident", name="ident")
            make_identity(nc, ident)

            # Crossing-block causal mask, multiplicative, replicated per head:
            # M2[p, h, j] = 1.0 iff j >= p.
            M2 = cst.tile([128, 2, 128], BF16, tag="mask", name="mask")
            nc.vector.memset(M2, 1.0)
            for h in range(2):
                nc.gpsimd.affine_select(
                    out=M2[:, h, :],
                    in_=M2[:, h, :],
                    compare_op=mybir.AluOpType.is_ge,
                    fill=0.0,
                    base=0,
                    pattern=[[1, 128]],
                    channel_multiplier=-1,
                )

            # ones row at partition 64 (bf16): stationary operand of the K=1
            # matmul that broadcasts the softmax denominator from partition 64
            # down to partitions 0..63.
            ones64r = cst.tile([128, 64], BF16, tag="ones64r", name="ones64r")
            nc.vector.memset(ones64r[64:65, :], 1.0)

            # ---- persistent activations ----
            QT = qkvt.tile([128, 2, T], BF16, tag="QT", name="QT")
            KT = qkvt.tile([128, 2, T], BF16, tag="KT", name="KT")
            VT = qkvt.tile([128, 2, T], BF16, tag="VT", name="VT")
            # V per hp-plane, token-major, per k-tile block of 130 cols:
            # [64 V_h0 | 1 | 64 V_h1 | 1]. memset 1.0 once; value columns are
            # overwritten by the transpose evacuations.
            V = qkvt.tile([128, 2, NKT, 130], BF16, tag="V", name="V")
            nc.vector.memset(V, 1.0)

            xts = [None] * NSLAB
            x_view = xT.rearrange("(ct p) t -> p ct t", p=128)

            def dma_slab(tj, split=False):
                xt = xin.tile([128, C // 128, 512], BF16, tag="xt", name="xt")
                if split:
                    # two halves so the first projection matmuls can start
                    # once their contraction chunks land (subtile deps), while
                    # keeping Sync-engine dispatch cost (~0.6us each) low
                    for h0 in (0, 4):
                        nc.sync.dma_start(
                            out=xt[:, h0 : h0 + 4, :],
                            in_=x_view[:, h0 : h0 + 4, tj * 512 : (tj + 1) * 512],
                        )
                else:
                    nc.sync.dma_start(
                        out=xt, in_=x_view[:, :, tj * 512 : (tj + 1) * 512]
                    )
                xts[tj] = xt

            # ---- weights / biases: wk + xt slab 0 first (they gate the
            # first projection); wp (only needed from chunk 1 on) last ----
            w_sb = {}
            b_sb = {}
            for name in ("k", "q", "v"):
                w_sb[name] = cst.tile(
                    [128, C // 128, FPC], BF16, tag=f"w{name}", name=f"w{name}"
                )
            wviews = {"k": wk, "q": wq, "v": wv}
            wk_view = wk.rearrange("(ct p) f -> p ct f", p=128)
            nc.sync.dma_start(out=w_sb["k"][:, 0:4, :], in_=wk_view[:, 0:4, :])
            dma_slab(0, split=True)
            nc.sync.dma_start(out=w_sb["k"][:, 4:8, :], in_=wk_view[:, 4:8, :])
            for name in ("q", "v"):
                nc.sync.dma_start(
                    out=w_sb[name],
                    in_=wviews[name].rearrange("(ct p) f -> p ct f", p=128),
                )
            for name, bap in (("k", bk), ("q", bq), ("v", bv)):
                b_sb[name] = cst.tile([128, 2], F32, tag=f"b{name}", name=f"b{name}")
                nc.sync.dma_start(out=b_sb[name], in_=bap)
            wp_sb = cst.tile([128, 2, C], BF16, tag="wp", name="wp")
            nc.sync.dma_start(
                out=wp_sb, in_=wp.rearrange("(hp p) f -> p hp f", p=128)
            )

            out_plane = {"q": QT, "k": KT, "v": VT}

            # ---- filler units: generators yielding once per PE instruction ----
            # The PE queue executes in order; to keep it continuously busy
            # (P-state ramps to 2.4 GHz only after ~3us without idle gaps)
            # filler work is drip-fed between attention k-tiles at single-
            # matmul granularity.

            def proj_unit(name, hp, tj):
                acc = ps.tile([128, 512], F32, tag="acc", name="acc")
                for ct in range(C // 128):
                    nc.tensor.matmul(
                        acc,
                        w_sb[name][:, ct, 128 * hp : 128 * hp + 128],
                        xts[tj][:, ct, :],
                        start=(ct == 0),
                        stop=(ct == C // 128 - 1),
                    )
                    yield
                nc.vector.tensor_scalar_add(
                    out_plane[name][:, hp, tj * 512 : (tj + 1) * 512],
                    acc,
                    b_sb[name][:, hp : hp + 1],
                )
                yield

            def transpose_unit(hp, tj):
                # 4 k-tiles' transposes batched into one PSUM tile + one copy
                pv4 = ps.tile([128, 4, 128], BF16, tag="acc", name="pv4")
                for j in range(4):
                    kt = 4 * tj + j
                    nc.tensor.transpose(
                        pv4[:, j, :], VT[:, hp, kt * 128 : (kt + 1) * 128], ident
                    )
                    yield
                nc.vector.tensor_copy(
                    out=V[:, hp, 4 * tj : 4 * tj + 4, :].rearrange(
                        "p k (s c) -> p k s c", c=65
                    )[:, :, :, 0:64],
                    in_=pv4.rearrange("p k (s c) -> p k s c", c=64),
                )
                yield

            def outproj_unit(qc, a0, a1t):
                q0 = qc * 512
                for tt in range(4):
                    yp = [
                        ps.tile([128, 512], F32, tag="acc", name="yp")
                        for _ in range(2)
                    ]
                    for cc in range(2):
                        nc.tensor.matmul(
                            yp[cc],
                            a0[:, tt * 128 : (tt + 1) * 128],
                            wp_sb[:, 0, cc * 512 : (cc + 1) * 512],
                            start=True,
                            stop=False,
                        )
                        yield
                    for cc in range(2):
                        nc.tensor.matmul(
                            yp[cc],
                            a1t[:, tt * 128 : (tt + 1) * 128],
                            wp_sb[:, 1, cc * 512 : (cc + 1) * 512],
                            start=False,
                            stop=True,
                        )
                        yield
                    ysb = yout.tile([128, C], BF16, tag="ysb", name="ysb")
                    nc.vector.tensor_copy(ysb[:, 0:512], yp[0])
                    nc.scalar.copy(ysb[:, 512:1024], yp[1])
                    t0 = q0 + tt * 128
                    nc.sync.dma_start(out=y[t0 : t0 + 128, :], in_=ysb)
                    yield

            def slab_units(tj, hps=(0, 1)):
                units = []
                for name in ("k", "q", "v"):
                    for hp in hps:
                        units.append((proj_unit(name, hp, tj), 9))
                for hp in hps:
                    units.append((transpose_unit(hp, tj), 5))
                return units

            # ---- per-chunk attention with software-pipelined S and filler ----
            def emit_s(hp, kt, qc):
                m = kt - 4 * qc
                w0 = 128 * m if m >= 0 else 0
                k0 = kt * 128
                q0 = qc * 512
                s = ps.tile([128, 2, 512], F32, tag="s", name="s")
                for h in range(2):
                    nc.tensor.matmul(
                        s[:, h, w0:512],
                        KT[64 * h : 64 * h + 64, hp, k0 : k0 + 128],
                        QT[64 * h : 64 * h + 64, hp, q0 + w0 : q0 + 512],
                        start=True,
                        stop=True,
                    )
                pt = ptile.tile([128, 2, 512], BF16, tag="pt", name="pt")
                nc.scalar.activation(
                    out=pt[:, :, w0:512], in_=s[:, :, w0:512], func=EXP, scale=SCALE
                )
                if m >= 0:
                    nc.gpsimd.tensor_mul(
                        pt[:, :, w0 : w0 + 128], pt[:, :, w0 : w0 + 128], M2
                    )
                return pt

            def outproj_half(plane, at, target, t0):
                # single-plane output projection partial (last chunk only)
                for tt in range(4):
                    yp = [
                        ps.tile([128, 512], F32, tag="acc", name="yph")
                        for _ in range(2)
                    ]
                    for cc in range(2):
                        nc.tensor.matmul(
                            yp[cc],
                            at[:, tt * 128 : (tt + 1) * 128],
                            wp_sb[:, plane, cc * 512 : (cc + 1) * 512],
                            start=True,
                            stop=True,
                        )
                        yield
                    ysb = yout.tile([128, C], BF16, tag="ysb", name="ysb")
                    nc.vector.tensor_copy(ysb[:, 0:512], yp[0])
                    nc.scalar.copy(ysb[:, 512:1024], yp[1])
                    r0 = t0 + tt * 128
                    nc.sync.dma_start(out=target[r0 : r0 + 128, :], in_=ysb)
                    yield

            def normalize(av, attn):
                for h in range(2):
                    denr = small.tile([128, 512], BF16, tag="denr", name="denr")
                    nc.vector.tensor_copy(out=denr[64:65, :], in_=av[h][64:65, :])
                    bc = ps.tile([64, 512], F32, tag="acc", name="bc")
                    nc.tensor.matmul(
                        bc, ones64r[64:65, :], denr[64:65, :], start=True, stop=True
                    )
                    rbc = small.tile([64, 512], F32, tag="rbc", name="rbc")
                    nc.vector.reciprocal_approx_fast(rbc, bc)
                    if h == 0:
                        nc.vector.tensor_mul(attn[0:64, :], av[h][0:64, :], rbc)
                    else:
                        a1 = attnp.tile([64, 512], BF16, tag="attn1", name="a1")
                        nc.vector.tensor_mul(a1, av[h][0:64, :], rbc)
                        # cross-partition move: only cheap path is DMA
                        nc.sync.dma_start(out=attn[64:128, :], in_=a1)

            # ---- prologue: slab 0 (hp0 eagerly; hp1 as chunk-0 filler) ----
            for gen, _ in slab_units(0, hps=(0,)):
                for _ in gen:
                    pass

            # Each slab's hp1 units are carried into the chunk that first
            # reads them, so every chunk — including the last, biggest one —
            # has filler to keep the PE busy while ScalarE drains exps.
            carry = slab_units(0, hps=(1,))
            op_carry = None
            guard_pre = 0
            carry2 = []
            for qc in range(NSLAB):
                carry_steps = sum(n for _, n in carry)
                pending = list(carry)
                # the deferred output projection is injected at the hp
                # boundary so it feeds the filler-starved hp1 phase
                late = [op_carry] if op_carry else []
                op_carry = None
                carry = []
                g_pre = guard_pre
                guard_pre = 0
                late2 = carry2
                carry2 = []
                if qc + 1 < NSLAB:
                    dma_slab(qc + 1)
                    if qc + 1 == NSLAB - 1:
                        # the last, biggest chunk needs the most filler, and
                        # its late hp1 phase needs it most. Units are carried
                        # to just before their first reader: k/v/transpose
                        # hp0 by k-tile 12 of the hp0 pass; q-hp1 by the hp
                        # boundary; k/v/transpose hp1 by k-tile 12 of the hp1
                        # pass. Only the hp0 q-projection (read by the
                        # chunk's first S) stays here.
                        pending += [(proj_unit("q", 0, qc + 1), 9)]
                        carry = [
                            (proj_unit("k", 0, qc + 1), 9),
                            (proj_unit("v", 0, qc + 1), 9),
                            (transpose_unit(0, qc + 1), 5),
                        ] + slab_units(qc + 1, hps=(1,))
                        guard_pre = 23
                    else:
                        pending += slab_units(qc + 1, hps=(0,))
                        carry = slab_units(qc + 1, hps=(1,))
                total_steps = sum(n for _, n in pending)
                nkt = 4 * (qc + 1)
                iters = [(hp, kt) for hp in range(2) for kt in range(nkt)]
                steps_done = 0

                def advance(target):
                    nonlocal steps_done, pending
                    while pending and steps_done < target:
                        gen, n = pending[0]
                        try:
                            next(gen)
                            steps_done += 1
                        except StopIteration:
                            pending.pop(0)

                av = {}
                pts = {0: emit_s(*iters[0], qc)}
                attn_tiles = []
                for i, (hp, kt) in enumerate(iters):
                    if i + 1 < len(iters):
                        if g_pre and i + 1 == nkt - 4:
                            # carried hp0 k/v/transpose units (front of
                            # `pending`) must be emitted before the k-tile-12
                            # S/AV matmuls that read them
                            advance(g_pre)
                        if i + 1 == nkt:
                            # this chunk's hp1 plane is produced by the
                            # carried units at the front of `pending` — they
                            # must be emitted before any reader (deps follow
                            # emission order)
                            advance(carry_steps)
                            pending.extend(late)
                            total_steps += sum(n for _, n in late)
                            late = []
                        pts[i + 1] = emit_s(*iters[i + 1], qc)
                    # filler goes between S(i+1) and AV(i) in the in-order PE
                    # queue: the PE chews filler while ScalarE finishes exp(i)
                    advance((i + 1) * total_steps // len(iters))
                    m = kt - 4 * qc
                    w0 = 128 * m if m >= 0 else 0
                    if kt == 0:
                        av[hp] = [
                            ps.tile([65, 512], F32, tag="av", name="av")
                            for _ in range(2)
                        ]
                    pt = pts.pop(i)
                    for h in range(2):
                        nc.tensor.matmul(
                            av[hp][h][:, w0:512],
                            V[:, hp, kt, 65 * h : 65 * h + 65],
                            pt[:, h, w0:512],
                            start=(kt == 0),
                            stop=(kt == nkt - 1),
                        )
                    if kt == nkt - 1:
                        attn = attnp.tile([128, 512], BF16, tag="attn", name="attn")
                        normalize(av[hp], attn)
                        attn_tiles.append(attn)
                        if qc == NSLAB - 1 and hp == 0:
                            # last chunk: hp0's projection half feeds the PE
                            # during the hp1 phase. Its steps MUST be counted
                            # in total_steps: the late2 guard targets are
                            # step counts over the front-ordered queue, and
                            # an uncounted unit ahead of late2 would absorb
                            # the guard budget and leave late2 unemitted
                            # before its readers.
                            pending.append(
                                (outproj_half(0, attn, y, qc * 512), 12)
                            )
                        # keep the PE fed while the DVE normalize chain frees
                        # the av buffers the next head-pair's AVs rotate onto
                        advance(steps_done + 6)

                while pending:
                    advance(steps_done + 100)

                if qc + 1 < NSLAB:
                    op_carry = (outproj_unit(qc, *attn_tiles), 20)
                else:
                    for _ in outproj_half(1, attn_tiles[1], y2, 0):
                        pass

    nc.compile()
    return nc


def _get_nc():
    if "nc" not in _CACHE:
        _CACHE["nc"] = _build()
    return _CACHE["nc"]


def _bf16(a):
    return np.ascontiguousarray(a.astype(ml_dtypes.bfloat16))


def _run(inputs, **spmd_kwargs):
    x = np.asarray(inputs["x"], dtype=np.float32)
    w_qkv = np.asarray(inputs["w_qkv"], dtype=np.float32)
    b_qkv = np.asarray(inputs["b_qkv"], dtype=np.float32)
    w_proj = np.asarray(inputs["w_proj"], dtype=np.float32)
    b_proj = np.asarray(inputs["b_proj"], dtype=np.float32)

    nc = _get_nc()

    in_maps = []
    for i in range(NCORES):
        b = i // NG
        g = i % NG
        f0 = g * FPC
        sl = slice(f0, f0 + FPC)
        in_maps.append(
            {
                "xT": _bf16(x[b].T),
                "wq": _bf16(w_qkv[:, sl]),
                "wk": _bf16(w_qkv[:, C + f0 : C + f0 + FPC]),
                "wv": _bf16(w_qkv[:, 2 * C + f0 : 2 * C + f0 + FPC]),
                "bq": np.ascontiguousarray(
                    b_qkv[sl].reshape(2, 128).T
                ),
                "bk": np.ascontiguousarray(
                    b_qkv[C + f0 : C + f0 + FPC].reshape(2, 128).T
                ),
                "bv": np.ascontiguousarray(
                    b_qkv[2 * C + f0 : 2 * C + f0 + FPC].reshape(2, 128).T
                ),
                "wp": _bf16(w_proj[sl, :]),
            }
        )

    res = run_bass_kernel_spmd(nc, in_maps, core_ids=list(range(NCORES)), **spmd_kwargs)
    acc = np.zeros((B, T, C), dtype=np.float64)
    for i, om in enumerate(res.results):
        acc[i // NG] += np.asarray(om["y"]).astype(np.float64)
        # last chunk's hp1 output-projection partial
        acc[i // NG, T - 512 :] += np.asarray(om["y2"]).astype(np.float64)
    out = (acc + b_proj.astype(np.float64)).astype(np.float32)
    return out, res


def kernel(**inputs) -> np.ndarray:
    out, _ = _run(inputs)
    return out


# revision 44
# speedup vs baseline: 1.0157x; 1.0024x over previous
"""Causal self-attention (B=2, T=2048, C=1024, H=16) on 8 TRN2 NeuronCores.

Sharding: (batch x head-group). Core (b, g) owns batch b and 4 heads
(2 head-pairs). It computes q/k/v projections for its 256 feature columns
over its batch's 2048 tokens, full causal attention for its (b, head) pairs,
and a partial output projection against its 256 rows of w_proj. The 4
partial [2048, 1024] outputs per batch are summed on host and b_proj is
added once during that reduction.

Within a core everything is "transposed" (features on partitions, tokens on
the free dim): xT [C, Tb] --PE--> Q^T/K^T/V^T planes [128, 2hp, 2048] and
V^T --PE transpose--> V (token-major, with an interleaved ones column per
head so softmax denominators fall out of the AV matmul).

All matmul operands are bf16: the PE streams 1 column/cycle regardless of
dtype but fp32 runs at half rate (fp32_mode=HIGH), so bf16 doubles matmul
throughput and enables fast weight loads. fp32 accumulation in PSUM
throughout; softmax denominators and reciprocals stay fp32. Softmax skips
the max-subtraction: scores are ~N(0,1) (bounded ~+-6), far inside exp
range.

Causal structure: per 512-wide q-chunk, diagonal k-tiles restrict the S
matmul / exp / AV matmul to the causally needed q-columns, and the mask
multiply (on the otherwise-idle GpSimd engine) touches only the 128-wide
crossing block.

Emission is software-pipelined: S matmuls run one k-tile ahead of the AV
matmuls, and independent filler work — the next token-slab's QKV projection
and V-transposes, plus the previous chunk's deferred output projection — is
drip-fed between S(i+1) and AV(i) at single-matmul granularity. This keeps
the in-order PE queue continuously busy while ScalarE (the only exp-capable
engine, ~1.1us per k-tile) paces the attention inner loop: PE idle gaps both
reset the P-state ramp (2.4 GHz only after ~3us of continuous execution) and
trip the HAM power manager down to half the array. Each slab's hp1 units are
carried into the chunk that first reads them, and the last chunk's output
projection is split per head-pair plane (the hp1 half lands in a separate
`y2` partial summed on host) so the kernel tail stays short.
"""

import numpy as np
import ml_dtypes

import concourse.bass as bass
import concourse.mybir as mybir
import concourse.tile as tile
from concourse import bacc
from concourse.bass_utils import run_bass_kernel_spmd
from concourse.masks import make_identity

F32 = mybir.dt.float32
F32R = mybir.dt.float32r
BF16 = mybir.dt.bfloat16
EXP = mybir.ActivationFunctionType.Exp

B, T, C = 2, 2048, 1024
H, DH = 16, 64
NCORES = 8
NG = 4                    # head-groups
FPC = 256                 # q/k/v feature columns per core (4 heads)
NKT = T // 128            # 16 k-tiles per batch
NSLAB = T // 512          # 4 token slabs / q-chunks
SCALE = DH ** -0.5

_CACHE = {}


def _build():
    nc = bacc.Bacc(
        "TRN2",
        target_bir_lowering=False,
        debug=False,
        enable_asserts=True,
        num_devices=NCORES,
    )
    xT = nc.dram_tensor("xT", [C, T], BF16, kind="ExternalInput").ap()
    wq = nc.dram_tensor("wq", [C, FPC], BF16, kind="ExternalInput").ap()
    wk = nc.dram_tensor("wk", [C, FPC], BF16, kind="ExternalInput").ap()
    wv = nc.dram_tensor("wv", [C, FPC], BF16, kind="ExternalInput").ap()
    bq = nc.dram_tensor("bq", [128, 2], F32, kind="ExternalInput").ap()
    bk = nc.dram_tensor("bk", [128, 2], F32, kind="ExternalInput").ap()
    bv = nc.dram_tensor("bv", [128, 2], F32, kind="ExternalInput").ap()
    wp = nc.dram_tensor("wp", [FPC, C], BF16, kind="ExternalInput").ap()
    y = nc.dram_tensor("y", [T, C], BF16, kind="ExternalOutput").ap()
    # last chunk's hp1 output-projection partial (host adds it into y's last
    # 512 rows) — lets the hp0 half run as filler instead of serializing the
    # whole projection after the final softmax
    y2 = nc.dram_tensor("y2", [512, C], BF16, kind="ExternalOutput").ap()

    with tile.TileContext(nc) as tc:
        with (
            tc.tile_pool(name="const", bufs=1) as cst,
            tc.tile_pool(name="qkvt", bufs=1) as qkvt,
            tc.tile_pool(name="xin", bufs=2) as xin,
            tc.tile_pool(name="ptile", bufs=3) as ptile,
            tc.tile_pool(name="attn", bufs=4) as attnp,
            tc.tile_pool(name="yout", bufs=4) as yout,
            tc.tile_pool(name="small", bufs=2) as small,
            tc.tile_pool(name="psum", bufs=2, space="PSUM") as ps,
        ):
            # ---- constants ----
            ident = cst.tile([128, 128], BF16, tag="ident", name="ident")
            make_identity(nc, ident)

            # Crossing-block causal mask, multiplicative, replicated per head:
            # M2[p, h, j] = 1.0 iff j >= p.
            M2 = cst.tile([128, 2, 128], BF16, tag="mask", name="mask")
            nc.vector.memset(M2, 1.0)
            for h in range(2):
                nc.gpsimd.affine_select(
                    out=M2[:, h, :],
                    in_=M2[:, h, :],
                    compare_op=mybir.AluOpType.is_ge,
                    fill=0.0,
                    base=0,
                    pattern=[[1, 128]],
                    channel_multiplier=-1,
                )

            # ones row at partition 64 (bf16): stationary operand of the K=1
            # matmul that broadcasts the softmax denominator from partition 64
            # down to partitions 0..63.
            ones64r = cst.tile([128, 64], BF16, tag="ones64r", name="ones64r")
            nc.vector.memset(ones64r[64:65, :], 1.0)

            # ---- persistent activations ----
            QT = qkvt.tile([128, 2, T], BF16, tag="QT", name="QT")
            KT = qkvt.tile([128, 2, T], BF16, tag="KT", name="KT")
            VT = qkvt.tile([128, 2, T], BF16, tag="VT", name="VT")
            # V per hp-plane, token-major, per k-tile block of 130 cols:
            # [64 V_h0 | 1 | 64 V_h1 | 1]. memset 1.0 once; value columns are
            # overwritten by the transpose evacuations.
            V = qkvt.tile([128, 2, NKT, 130], BF16, tag="V", name="V")
            nc.vector.memset(V, 1.0)

            xts = [None] * NSLAB
            x_view = xT.rearrange("(ct p) t -> p ct t", p=128)

            def dma_slab(tj, split=False):
                xt = xin.tile([128, C // 128, 512], BF16, tag="xt", name="xt")
                if split:
                    # two halves so the first projection matmuls can start
                    # once their contraction chunks land (subtile deps), while
                    # keeping Sync-engine dispatch cost (~0.6us each) low
                    for h0 in (0, 4):
                        nc.sync.dma_start(
                            out=xt[:, h0 : h0 + 4, :],
                            in_=x_view[:, h0 : h0 + 4, tj * 512 : (tj + 1) * 512],
                        )
                else:
                    nc.sync.dma_start(
                        out=xt, in_=x_view[:, :, tj * 512 : (tj + 1) * 512]
                    )
                xts[tj] = xt

            # ---- weights / biases: wk + xt slab 0 first (they gate the
            # first projection); wp (only needed from chunk 1 on) last ----
            w_sb = {}
            b_sb = {}
            for name in ("k", "q", "v"):
                w_sb[name] = cst.tile(
                    [128, C // 128, FPC], BF16, tag=f"w{name}", name=f"w{name}"
                )
            wviews = {"k": wk, "q": wq, "v": wv}
            wk_view = wk.rearrange("(ct p) f -> p ct f", p=128)
            nc.sync.dma_start(out=w_sb["k"][:, 0:4, :], in_=wk_view[:, 0:4, :])
            dma_slab(0, split=True)
            nc.sync.dma_start(out=w_sb["k"][:, 4:8, :], in_=wk_view[:, 4:8, :])
            for name in ("q", "v"):
                nc.sync.dma_start(
                    out=w_sb[name],
                    in_=wviews[name].rearrange("(ct p) f -> p ct f", p=128),
                )
            for name, bap in (("k", bk), ("q", bq), ("v", bv)):
                b_sb[name] = cst.tile([128, 2], F32, tag=f"b{name}", name=f"b{name}")
                nc.sync.dma_start(out=b_sb[name], in_=bap)
            wp_sb = cst.tile([128, 2, C], BF16, tag="wp", name="wp")
            nc.sync.dma_start(
                out=wp_sb, in_=wp.rearrange("(hp p) f -> p hp f", p=128)
            )

            out_plane = {"q": QT, "k": KT, "v": VT}

            # ---- filler units: generators yielding once per PE instruction ----
            # The PE queue executes in order; to keep it continuously busy
            # (P-state ramps to 2.4 GHz only after ~3us without idle gaps)
            # filler work is drip-fed between attention k-tiles at single-
            # matmul granularity.

            def proj_unit(name, hp, tj):
                acc = ps.tile([128, 512], F32, tag="acc", name="acc")
                for ct in range(C // 128):
                    nc.tensor.matmul(
                        acc,
                        w_sb[name][:, ct, 128 * hp : 128 * hp + 128],
                        xts[tj][:, ct, :],
                        start=(ct == 0),
                        stop=(ct == C // 128 - 1),
                    )
                    yield
                nc.vector.tensor_scalar_add(
                    out_plane[name][:, hp, tj * 512 : (tj + 1) * 512],
                    acc,
                    b_sb[name][:, hp : hp + 1],
                )
                yield

            def transpose_unit(hp, tj):
                # 4 k-tiles' transposes batched into one PSUM tile + one copy
                pv4 = ps.tile([128, 4, 128], BF16, tag="acc", name="pv4")
                for j in range(4):
                    kt = 4 * tj + j
                    nc.tensor.transpose(
                        pv4[:, j, :], VT[:, hp, kt * 128 : (kt + 1) * 128], ident
                    )
                    yield
                nc.vector.tensor_copy(
                    out=V[:, hp, 4 * tj : 4 * tj + 4, :].rearrange(
                        "p k (s c) -> p k s c", c=65
                    )[:, :, :, 0:64],
                    in_=pv4.rearrange("p k (s c) -> p k s c", c=64),
                )
                yield

            def outproj_unit(qc, a0, a1t):
                q0 = qc * 512
                for tt in range(4):
                    yp = [
                        ps.tile([128, 512], F32, tag="acc", name="yp")
                        for _ in range(2)
                    ]
                    for cc in range(2):
                        nc.tensor.matmul(
                            yp[cc],
                            a0[:, tt * 128 : (tt + 1) * 128],
                            wp_sb[:, 0, cc * 512 : (cc + 1) * 512],
                            start=True,
                            stop=False,
                        )
                        yield
                    for cc in range(2):
                        nc.tensor.matmul(
                            yp[cc],
                            a1t[:, tt * 128 : (tt + 1) * 128],
                            wp_sb[:, 1, cc * 512 : (cc + 1) * 512],
                            start=False,
                            stop=True,
                        )
                        yield
                    ysb = yout.tile([128, C], BF16, tag="ysb", name="ysb")
                    nc.vector.tensor_copy(ysb[:, 0:512], yp[0])
                    nc.scalar.copy(ysb[:, 512:1024], yp[1])
                    t0 = q0 + tt * 128
                    nc.sync.dma_start(out=y[t0 : t0 + 128, :], in_=ysb)
                    yield

            def slab_units(tj, hps=(0, 1)):
                units = []
                for name in ("k", "q", "v"):
                    for hp in hps:
                        units.append((proj_unit(name, hp, tj), 9))
                for hp in hps:
                    units.append((transpose_unit(hp, tj), 5))
                return units

            # ---- per-chunk attention with software-pipelined S and filler ----
            def emit_s(hp, kt, qc):
                m = kt - 4 * qc
                w0 = 128 * m if m >= 0 else 0
                k0 = kt * 128
                q0 = qc * 512
                s = ps.tile([128, 2, 512], F32, tag="s", name="s")
                for h in range(2):
                    nc.tensor.matmul(
                        s[:, h, w0:512],
                        KT[64 * h : 64 * h + 64, hp, k0 : k0 + 128],
                        QT[64 * h : 64 * h + 64, hp, q0 + w0 : q0 + 512],
                        start=True,
                        stop=True,
                    )
                pt = ptile.tile([128, 2, 512], BF16, tag="pt", name="pt")
                nc.scalar.activation(
                    out=pt[:, :, w0:512], in_=s[:, :, w0:512], func=EXP, scale=SCALE
                )
                if m >= 0:
                    nc.gpsimd.tensor_mul(
                        pt[:, :, w0 : w0 + 128], pt[:, :, w0 : w0 + 128], M2
                    )
                return pt

            def outproj_half(plane, at, target, t0):
                # single-plane output projection partial (last chunk only)
                for tt in range(4):
                    yp = [
                        ps.tile([128, 512], F32, tag="acc", name="yph")
                        for _ in range(2)
                    ]
                    for cc in range(2):
                        nc.tensor.matmul(
                            yp[cc],
                            at[:, tt * 128 : (tt + 1) * 128],
                            wp_sb[:, plane, cc * 512 : (cc + 1) * 512],
                            start=True,
                            stop=True,
                        )
                        yield
                    ysb = yout.tile([128, C], BF16, tag="ysb", name="ysb")
                    nc.vector.tensor_copy(ysb[:, 0:512], yp[0])
                    nc.scalar.copy(ysb[:, 512:1024], yp[1])
                    r0 = t0 + tt * 128
                    nc.sync.dma_start(out=target[r0 : r0 + 128, :], in_=ysb)
                    yield

            def normalize(av, attn):
                for h in range(2):
                    denr = small.tile([128, 512], BF16, tag="denr", name="denr")
                    nc.vector.tensor_copy(out=denr[64:65, :], in_=av[h][64:65, :])
                    bc = ps.tile([64, 512], F32, tag="acc", name="bc")
                    nc.tensor.matmul(
                        bc, ones64r[64:65, :], denr[64:65, :], start=True, stop=True
                    )
                    rbc = small.tile([64, 512], F32, tag="rbc", name="rbc")
                    nc.vector.reciprocal_approx_fast(rbc, bc)
                    if h == 0:
                        nc.vector.tensor_mul(attn[0:64, :], av[h][0:64, :], rbc)
                    else:
                        a1 = attnp.tile([64, 512], BF16, tag="attn1", name="a1")
                        nc.vector.tensor_mul(a1, av[h][0:64, :], rbc)
                        # cross-partition move: only cheap path is DMA
                        nc.sync.dma_start(out=attn[64:128, :], in_=a1)

            # ---- prologue: slab 0 (hp0 eagerly; hp1 as chunk-0 filler) ----
            for gen, _ in slab_units(0, hps=(0,)):
                for _ in gen:
                    pass

            # Each slab's hp1 units are carried into the chunk that first
            # reads them, so every chunk — including the last, biggest one —
            # has filler to keep the PE busy while ScalarE drains exps.
            carry = slab_units(0, hps=(1,))
            op_carry = None
            guard_pre = 0
            carry2 = []
            for qc in range(NSLAB):
                carry_steps = sum(n for _, n in carry)
                pending = list(carry)
                # the deferred output projection is injected at the hp
                # boundary so it feeds the filler-starved hp1 phase
                late = [op_carry] if op_carry else []
                op_carry = None
                carry = []
                g_pre = guard_pre
                guard_pre = 0
                late2 = carry2
                carry2 = []
                if qc + 1 < NSLAB:
                    dma_slab(qc + 1)
                    if qc + 1 == NSLAB - 1:
                        # the last, biggest chunk needs the most filler, and
                        # its late hp1 phase needs it most. Units are carried
                        # to just before their first reader: k/v/transpose
                        # hp0 by k-tile 12 of the hp0 pass; q-hp1 by the hp
                        # boundary; k/v/transpose hp1 by k-tile 12 of the hp1
                        # pass. Only the hp0 q-projection (read by the
                        # chunk's first S) stays here.
                        pending += [(proj_unit("q", 0, qc + 1), 9)]
                        carry = [
                            (proj_unit("k", 0, qc + 1), 9),
                            (proj_unit("v", 0, qc + 1), 9),
                            (transpose_unit(0, qc + 1), 5),
                        ] + slab_units(qc + 1, hps=(1,))
                        guard_pre = 23
                    else:
                        pending += slab_units(qc + 1, hps=(0,))
                        carry = slab_units(qc + 1, hps=(1,))
                total_steps = sum(n for _, n in pending)
                nkt = 4 * (qc + 1)
                iters = [(hp, kt) for hp in range(2) for kt in range(nkt)]
                steps_done = 0

                def advance(target):
                    nonlocal steps_done, pending
                    while pending and steps_done < target:
                        gen, n = pending[0]
                        try:
                            next(gen)
                            steps_done += 1
                        except StopIteration:
                            pending.pop(0)

                av = {}
                pts = {0: emit_s(*iters[0], qc)}
                attn_tiles = []
                for i, (hp, kt) in enumerate(iters):
                    if i + 1 < len(iters):
                        if g_pre and i + 1 == nkt - 4:
                            # carried hp0 k/v/transpose units (front of
                            # `pending`) must be emitted before the k-tile-12
                            # S/AV matmuls that read them
                            advance(g_pre)
                        if i + 1 == nkt:
                            # this chunk's hp1 plane is produced by the
                            # carried units at the front of `pending` — they
                            # must be emitted before any reader (deps follow
                            # emission order)
                            advance(carry_steps)
                            pending.extend(late)
                            total_steps += sum(n for _, n in late)
                            late = []
                        pts[i + 1] = emit_s(*iters[i + 1], qc)
                    # filler goes between S(i+1) and AV(i) in the in-order PE
                    # queue: the PE chews filler while ScalarE finishes exp(i)
                    advance((i + 1) * total_steps // len(iters))
                    m = kt - 4 * qc
                    w0 = 128 * m if m >= 0 else 0
                    if kt == 0:
                        av[hp] = [
                            ps.tile([65, 512], F32, tag="av", name="av")
                            for _ in range(2)
                        ]
                    pt = pts.pop(i)
                    for h in range(2):
                        nc.tensor.matmul(
                            av[hp][h][:, w0:512],
                            V[:, hp, kt, 65 * h : 65 * h + 65],
                            pt[:, h, w0:512],
                            start=(kt == 0),
                            stop=(kt == nkt - 1),
                        )
                    if kt == nkt - 1:
                        attn = attnp.tile([128, 512], BF16, tag="attn", name="attn")
                        normalize(av[hp], attn)
                        attn_tiles.append(attn)
                        if qc == NSLAB - 1 and hp == 0:
                            # last chunk: hp0's projection half feeds the PE
                            # during the hp1 phase. Its steps MUST be counted
                            # in total_steps: the late2 guard targets are
                            # step counts over the front-ordered queue, and
                            # an uncounted unit ahead of late2 would absorb
                            # the guard budget and leave late2 unemitted
                            # before its readers.
                            pending.append(
                                (outproj_half(0, attn, y, qc * 512), 12)
                            )
                        # keep the PE fed while the DVE normalize chain frees
                        # the av buffers the next head-pair's AVs rotate onto
                        advance(steps_done + 10)

                while pending:
                    advance(steps_done + 100)

                if qc + 1 < NSLAB:
                    op_carry = (outproj_unit(qc, *attn_tiles), 20)
                else:
                    for _ in outproj_half(1, attn_tiles[1], y2, 0):
                        pass

    nc.compile()
    return nc


def _get_nc():
    if "nc" not in _CACHE:
        _CACHE["nc"] = _build()
    return _CACHE["nc"]


def _bf16(a):
    return np.ascontiguousarray(a.astype(ml_dtypes.bfloat16))


def _run(inputs, **spmd_kwargs):
    x = np.asarray(inputs["x"], dtype=np.float32)
    w_qkv = np.asarray(inputs["w_qkv"], dtype=np.float32)
    b_qkv = np.asarray(inputs["b_qkv"], dtype=np.float32)
    w_proj = np.asarray(inputs["w_proj"], dtype=np.float32)
    b_proj = np.asarray(inputs["b_proj"], dtype=np.float32)

    nc = _get_nc()

    in_maps = []
    for i in range(NCORES):
        b = i // NG
        g = i % NG
        f0 = g * FPC
        sl = slice(f0, f0 + FPC)
        in_maps.append(
            {
                "xT": _bf16(x[b].T),
                "wq": _bf16(w_qkv[:, sl]),
                "wk": _bf16(w_qkv[:, C + f0 : C + f0 + FPC]),
                "wv": _bf16(w_qkv[:, 2 * C + f0 : 2 * C + f0 + FPC]),
                "bq": np.ascontiguousarray(
                    b_qkv[sl].reshape(2, 128).T
                ),
                "bk": np.ascontiguousarray(
                    b_qkv[C + f0 : C + f0 + FPC].reshape(2, 128).T
                ),
                "bv": np.ascontiguousarray(
                    b_qkv[2 * C + f0 : 2 * C + f0 + FPC].reshape(2, 128).T
                ),
                "wp": _bf16(w_proj[sl, :]),
            }
        )

    res = run_bass_kernel_spmd(nc, in_maps, core_ids=list(range(NCORES)), **spmd_kwargs)
    acc = np.zeros((B, T, C), dtype=np.float64)
    for i, om in enumerate(res.results):
        acc[i // NG] += np.asarray(om["y"]).astype(np.float64)
        # last chunk's hp1 output-projection partial
        acc[i // NG, T - 512 :] += np.asarray(om["y2"]).astype(np.float64)
    out = (acc + b_proj.astype(np.float64)).astype(np.float32)
    return out, res


def kernel(**inputs) -> np.ndarray:
    out, _ = _run(inputs)
    return out


# revision 47
# speedup vs baseline: 1.0159x; 1.0003x over previous
"""Causal self-attention (B=2, T=2048, C=1024, H=16) on 8 TRN2 NeuronCores.

Sharding: (batch x head-group). Core (b, g) owns batch b and 4 heads
(2 head-pairs). It computes q/k/v projections for its 256 feature columns
over its batch's 2048 tokens, full causal attention for its (b, head) pairs,
and a partial output projection against its 256 rows of w_proj. The 4
partial [2048, 1024] outputs per batch are summed on host and b_proj is
added once during that reduction.

Within a core everything is "transposed" (features on partitions, tokens on
the free dim): xT [C, Tb] --PE--> Q^T/K^T/V^T planes [128, 2hp, 2048] and
V^T --PE transpose--> V (token-major, with an interleaved ones column per
head so softmax denominators fall out of the AV matmul).

All matmul operands are bf16: the PE streams 1 column/cycle regardless of
dtype but fp32 runs at half rate (fp32_mode=HIGH), so bf16 doubles matmul
throughput and enables fast weight loads. fp32 accumulation in PSUM
throughout; softmax denominators and reciprocals stay fp32. Softmax skips
the max-subtraction: scores are ~N(0,1) (bounded ~+-6), far inside exp
range.

Causal structure: per 512-wide q-chunk, diagonal k-tiles restrict the S
matmul / exp / AV matmul to the causally needed q-columns, and the mask
multiply (on the otherwise-idle GpSimd engine) touches only the 128-wide
crossing block.

Emission is software-pipelined: S matmuls run one k-tile ahead of the AV
matmuls, and independent filler work — the next token-slab's QKV projection
and V-transposes, plus the previous chunk's deferred output projection — is
drip-fed between S(i+1) and AV(i) at single-matmul granularity. This keeps
the in-order PE queue continuously busy while ScalarE (the only exp-capable
engine, ~1.1us per k-tile) paces the attention inner loop: PE idle gaps both
reset the P-state ramp (2.4 GHz only after ~3us of continuous execution) and
trip the HAM power manager down to half the array. Each slab's hp1 units are
carried into the chunk that first reads them, and the last chunk's output
projection is split per head-pair plane (the hp1 half lands in a separate
`y2` partial summed on host) so the kernel tail stays short.
"""

import numpy as np
import ml_dtypes

import concourse.bass as bass
import concourse.mybir as mybir
import concourse.tile as tile
from concourse import bacc
from concourse.bass_utils import run_bass_kernel_spmd
from concourse.masks import make_identity

F32 = mybir.dt.float32
F32R = mybir.dt.float32r
BF16 = mybir.dt.bfloat16
EXP = mybir.ActivationFunctionType.Exp

B, T, C = 2, 2048, 1024
H, DH = 16, 64
NCORES = 8
NG = 4                    # head-groups
FPC = 256                 # q/k/v feature columns per core (4 heads)
NKT = T // 128            # 16 k-tiles per batch
NSLAB = T // 512          # 4 token slabs / q-chunks
SCALE = DH ** -0.5

_CACHE = {}


def _build():
    nc = bacc.Bacc(
        "TRN2",
        target_bir_lowering=False,
        debug=False,
        enable_asserts=True,
        num_devices=NCORES,
    )
    xT = nc.dram_tensor("xT", [C, T], BF16, kind="ExternalInput").ap()
    wq = nc.dram_tensor("wq", [C, FPC], BF16, kind="ExternalInput").ap()
    wk = nc.dram_tensor("wk", [C, FPC], BF16, kind="ExternalInput").ap()
    wv = nc.dram_tensor("wv", [C, FPC], BF16, kind="ExternalInput").ap()
    bq = nc.dram_tensor("bq", [128, 2], F32, kind="ExternalInput").ap()
    bk = nc.dram_tensor("bk", [128, 2], F32, kind="ExternalInput").ap()
    bv = nc.dram_tensor("bv", [128, 2], F32, kind="ExternalInput").ap()
    wp = nc.dram_tensor("wp", [FPC, C], BF16, kind="ExternalInput").ap()
    y = nc.dram_tensor("y", [T, C], BF16, kind="ExternalOutput").ap()
    # last chunk's hp1 output-projection partial (host adds it into y's last
    # 512 rows) — lets the hp0 half run as filler instead of serializing the
    # whole projection after the final softmax
    y2 = nc.dram_tensor("y2", [512, C], BF16, kind="ExternalOutput").ap()

    with tile.TileContext(nc) as tc:
        with (
            tc.tile_pool(name="const", bufs=1) as cst,
            tc.tile_pool(name="qkvt", bufs=1) as qkvt,
            tc.tile_pool(name="xin", bufs=2) as xin,
            tc.tile_pool(name="ptile", bufs=3) as ptile,
            tc.tile_pool(name="attn", bufs=4) as attnp,
            tc.tile_pool(name="yout", bufs=4) as yout,
            tc.tile_pool(name="small", bufs=2) as small,
            tc.tile_pool(name="psum", bufs=2, space="PSUM") as ps,
        ):
            # ---- constants ----
            ident = cst.tile([128, 128], BF16, tag="ident", name="ident")
            make_identity(nc, ident)

            # Crossing-block causal mask, multiplicative, replicated per head:
            # M2[p, h, j] = 1.0 iff j >= p.
            M2 = cst.tile([128, 2, 128], BF16, tag="mask", name="mask")
            nc.vector.memset(M2, 1.0)
            for h in range(2):
                nc.gpsimd.affine_select(
                    out=M2[:, h, :],
                    in_=M2[:, h, :],
                    compare_op=mybir.AluOpType.is_ge,
                    fill=0.0,
                    base=0,
                    pattern=[[1, 128]],
                    channel_multiplier=-1,
                )

            # ones row at partition 64 (bf16): stationary operand of the K=1
            # matmul that broadcasts the softmax denominator from partition 64
            # down to partitions 0..63.
            ones64r = cst.tile([128, 64], BF16, tag="ones64r", name="ones64r")
            nc.vector.memset(ones64r[64:65, :], 1.0)

            # ---- persistent activations ----
            QT = qkvt.tile([128, 2, T], BF16, tag="QT", name="QT")
            KT = qkvt.tile([128, 2, T], BF16, tag="KT", name="KT")
            VT = qkvt.tile([128, 2, T], BF16, tag="VT", name="VT")
            # V per hp-plane, token-major, per k-tile block of 130 cols:
            # [64 V_h0 | 1 | 64 V_h1 | 1]. memset 1.0 once; value columns are
            # overwritten by the transpose evacuations.
            V = qkvt.tile([128, 2, NKT, 130], BF16, tag="V", name="V")
            nc.vector.memset(V, 1.0)

            xts = [None] * NSLAB
            x_view = xT.rearrange("(ct p) t -> p ct t", p=128)

            def dma_slab(tj, split=False):
                xt = xin.tile([128, C // 128, 512], BF16, tag="xt", name="xt")
                if split:
                    # two halves so the first projection matmuls can start
                    # once their contraction chunks land (subtile deps), while
                    # keeping Sync-engine dispatch cost (~0.6us each) low
                    for h0 in (0, 4):
                        nc.sync.dma_start(
                            out=xt[:, h0 : h0 + 4, :],
                            in_=x_view[:, h0 : h0 + 4, tj * 512 : (tj + 1) * 512],
                        )
                else:
                    nc.sync.dma_start(
                        out=xt, in_=x_view[:, :, tj * 512 : (tj + 1) * 512]
                    )
                xts[tj] = xt

            # ---- weights / biases: wk + xt slab 0 first (they gate the
            # first projection); wp (only needed from chunk 1 on) last ----
            w_sb = {}
            b_sb = {}
            for name in ("k", "q", "v"):
                w_sb[name] = cst.tile(
                    [128, C // 128, FPC], BF16, tag=f"w{name}", name=f"w{name}"
                )
            wviews = {"k": wk, "q": wq, "v": wv}
            wk_view = wk.rearrange("(ct p) f -> p ct f", p=128)
            nc.sync.dma_start(out=w_sb["k"][:, 0:4, :], in_=wk_view[:, 0:4, :])
            dma_slab(0, split=True)
            nc.sync.dma_start(out=w_sb["k"][:, 4:8, :], in_=wk_view[:, 4:8, :])
            for name in ("q", "v"):
                nc.sync.dma_start(
                    out=w_sb[name],
                    in_=wviews[name].rearrange("(ct p) f -> p ct f", p=128),
                )
            for name, bap in (("k", bk), ("q", bq), ("v", bv)):
                b_sb[name] = cst.tile([128, 2], F32, tag=f"b{name}", name=f"b{name}")
                nc.sync.dma_start(out=b_sb[name], in_=bap)
            wp_sb = cst.tile([128, 2, C], BF16, tag="wp", name="wp")
            nc.sync.dma_start(
                out=wp_sb, in_=wp.rearrange("(hp p) f -> p hp f", p=128)
            )

            out_plane = {"q": QT, "k": KT, "v": VT}

            # ---- filler units: generators yielding once per PE instruction ----
            # The PE queue executes in order; to keep it continuously busy
            # (P-state ramps to 2.4 GHz only after ~3us without idle gaps)
            # filler work is drip-fed between attention k-tiles at single-
            # matmul granularity.

            def proj_unit(name, hp, tj):
                acc = ps.tile([128, 512], F32, tag="acc", name="acc")
                for ct in range(C // 128):
                    nc.tensor.matmul(
                        acc,
                        w_sb[name][:, ct, 128 * hp : 128 * hp + 128],
                        xts[tj][:, ct, :],
                        start=(ct == 0),
                        stop=(ct == C // 128 - 1),
                    )
                    yield
                nc.vector.tensor_scalar_add(
                    out_plane[name][:, hp, tj * 512 : (tj + 1) * 512],
                    acc,
                    b_sb[name][:, hp : hp + 1],
                )
                yield

            def transpose_unit(hp, tj):
                # 4 k-tiles' transposes batched into one PSUM tile + one copy
                pv4 = ps.tile([128, 4, 128], BF16, tag="acc", name="pv4")
                for j in range(4):
                    kt = 4 * tj + j
                    nc.tensor.transpose(
                        pv4[:, j, :], VT[:, hp, kt * 128 : (kt + 1) * 128], ident
                    )
                    yield
                nc.vector.tensor_copy(
                    out=V[:, hp, 4 * tj : 4 * tj + 4, :].rearrange(
                        "p k (s c) -> p k s c", c=65
                    )[:, :, :, 0:64],
                    in_=pv4.rearrange("p k (s c) -> p k s c", c=64),
                )
                yield

            def outproj_unit(qc, a0, a1t):
                q0 = qc * 512
                for tt in range(4):
                    yp = [
                        ps.tile([128, 512], F32, tag="acc", name="yp")
                        for _ in range(2)
                    ]
                    for cc in range(2):
                        nc.tensor.matmul(
                            yp[cc],
                            a0[:, tt * 128 : (tt + 1) * 128],
                            wp_sb[:, 0, cc * 512 : (cc + 1) * 512],
                            start=True,
                            stop=False,
                        )
                        yield
                    for cc in range(2):
                        nc.tensor.matmul(
                            yp[cc],
                            a1t[:, tt * 128 : (tt + 1) * 128],
                            wp_sb[:, 1, cc * 512 : (cc + 1) * 512],
                            start=False,
                            stop=True,
                        )
                        yield
                    ysb = yout.tile([128, C], BF16, tag="ysb", name="ysb")
                    nc.vector.tensor_copy(ysb[:, 0:512], yp[0])
                    nc.scalar.copy(ysb[:, 512:1024], yp[1])
                    t0 = q0 + tt * 128
                    nc.sync.dma_start(out=y[t0 : t0 + 128, :], in_=ysb)
                    yield

            def slab_units(tj, hps=(0, 1)):
                units = []
                for name in ("k", "q", "v"):
                    for hp in hps:
                        units.append((proj_unit(name, hp, tj), 9))
                for hp in hps:
                    units.append((transpose_unit(hp, tj), 5))
                return units

            # ---- per-chunk attention with software-pipelined S and filler ----
            def emit_s(hp, kt, qc):
                m = kt - 4 * qc
                w0 = 128 * m if m >= 0 else 0
                k0 = kt * 128
                q0 = qc * 512
                s = ps.tile([128, 2, 512], F32, tag="s", name="s")
                for h in range(2):
                    nc.tensor.matmul(
                        s[:, h, w0:512],
                        KT[64 * h : 64 * h + 64, hp, k0 : k0 + 128],
                        QT[64 * h : 64 * h + 64, hp, q0 + w0 : q0 + 512],
                        start=True,
                        stop=True,
                    )
                pt = ptile.tile([128, 2, 512], BF16, tag="pt", name="pt")
                nc.scalar.activation(
                    out=pt[:, :, w0:512], in_=s[:, :, w0:512], func=EXP, scale=SCALE
                )
                if m >= 0:
                    nc.gpsimd.tensor_mul(
                        pt[:, :, w0 : w0 + 128], pt[:, :, w0 : w0 + 128], M2
                    )
                return pt

            def outproj_half(plane, at, target, t0):
                # single-plane output projection partial (last chunk only)
                for tt in range(4):
                    yp = [
                        ps.tile([128, 512], F32, tag="acc", name="yph")
                        for _ in range(2)
                    ]
                    for cc in range(2):
                        nc.tensor.matmul(
                            yp[cc],
                            at[:, tt * 128 : (tt + 1) * 128],
                            wp_sb[:, plane, cc * 512 : (cc + 1) * 512],
                            start=True,
                            stop=True,
                        )
                        yield
                    ysb = yout.tile([128, C], BF16, tag="ysb", name="ysb")
                    nc.vector.tensor_copy(ysb[:, 0:512], yp[0])
                    nc.scalar.copy(ysb[:, 512:1024], yp[1])
                    r0 = t0 + tt * 128
                    nc.sync.dma_start(out=target[r0 : r0 + 128, :], in_=ysb)
                    yield

            def normalize(av, attn):
                for h in range(2):
                    denr = small.tile([128, 512], BF16, tag="denr", name="denr")
                    nc.vector.tensor_copy(out=denr[64:65, :], in_=av[h][64:65, :])
                    bc = ps.tile([64, 512], F32, tag="acc", name="bc")
                    nc.tensor.matmul(
                        bc, ones64r[64:65, :], denr[64:65, :], start=True, stop=True
                    )
                    rbc = small.tile([64, 512], F32, tag="rbc", name="rbc")
                    nc.vector.reciprocal_approx_fast(rbc, bc)
                    if h == 0:
                        nc.vector.tensor_mul(attn[0:64, :], av[h][0:64, :], rbc)
                    else:
                        a1 = attnp.tile([64, 512], BF16, tag="attn1", name="a1")
                        nc.vector.tensor_mul(a1, av[h][0:64, :], rbc)
                        # cross-partition move: only cheap path is DMA
                        nc.sync.dma_start(out=attn[64:128, :], in_=a1)

            # ---- prologue: slab 0 (hp0 eagerly; hp1 as chunk-0 filler) ----
            for gen, _ in slab_units(0, hps=(0,)):
                for _ in gen:
                    pass

            # Each slab's hp1 units are carried into the chunk that first
            # reads them, so every chunk — including the last, biggest one —
            # has filler to keep the PE busy while ScalarE drains exps.
            carry = slab_units(0, hps=(1,))
            op_carry = None
            guard_pre = 0
            for qc in range(NSLAB):
                carry_steps = sum(n for _, n in carry)
                pending = list(carry)
                # the deferred output projection is injected at the hp
                # boundary so it feeds the filler-starved hp1 phase
                late = [op_carry] if op_carry else []
                op_carry = None
                carry = []
                g_pre = guard_pre
                guard_pre = 0
                if qc + 1 < NSLAB:
                    dma_slab(qc + 1)
                    if qc + 1 == NSLAB - 1:
                        # the last, biggest chunk needs the most filler, and
                        # its late hp1 phase needs it most. Units are carried
                        # to just before their first reader: k/v/transpose
                        # hp0 by k-tile 12 of the hp0 pass; q-hp1 by the hp
                        # boundary; k/v/transpose hp1 by k-tile 12 of the hp1
                        # pass. Only the hp0 q-projection (read by the
                        # chunk's first S) stays here.
                        pending += [(proj_unit("q", 0, qc + 1), 9)]
                        carry = [
                            (proj_unit("k", 0, qc + 1), 9),
                            (proj_unit("v", 0, qc + 1), 9),
                            (transpose_unit(0, qc + 1), 5),
                        ] + slab_units(qc + 1, hps=(1,))
                        guard_pre = 23
                    else:
                        pending += slab_units(qc + 1, hps=(0,))
                        carry = slab_units(qc + 1, hps=(1,))
                total_steps = sum(n for _, n in pending)
                nkt = 4 * (qc + 1)
                iters = [(hp, kt) for hp in range(2) for kt in range(nkt)]
                steps_done = 0

                def advance(target):
                    nonlocal steps_done, pending
                    while pending and steps_done < target:
                        gen, n = pending[0]
                        try:
                            next(gen)
                            steps_done += 1
                        except StopIteration:
                            pending.pop(0)

                av = {}
                pts = {0: emit_s(*iters[0], qc)}
                attn_tiles = []
                for i, (hp, kt) in enumerate(iters):
                    if i + 1 < len(iters):
                        if g_pre and i + 1 == nkt - 4:
                            # carried hp0 k/v/transpose units (front of
                            # `pending`) must be emitted before the k-tile-12
                            # S/AV matmuls that read them
                            advance(g_pre)
                        if i + 1 == nkt:
                            # this chunk's hp1 plane is produced by the
                            # carried units at the front of `pending` — they
                            # must be emitted before any reader (deps follow
                            # emission order)
                            advance(carry_steps)
                            pending.extend(late)
                            total_steps += sum(n for _, n in late)
                            late = []
                            # the guards leave steps_done above the pacing
                            # targets, which would otherwise emit no filler
                            # for the first ~8 hp1 iterations — exactly the
                            # measured starvation window that trips HAM
                            advance(steps_done + 8)
                        pts[i + 1] = emit_s(*iters[i + 1], qc)
                    # filler goes between S(i+1) and AV(i) in the in-order PE
                    # queue: the PE chews filler while ScalarE finishes exp(i)
                    advance((i + 1) * total_steps // len(iters))
                    m = kt - 4 * qc
                    w0 = 128 * m if m >= 0 else 0
                    if kt == 0:
                        av[hp] = [
                            ps.tile([65, 512], F32, tag="av", name="av")
                            for _ in range(2)
                        ]
                    pt = pts.pop(i)
                    for h in range(2):
                        nc.tensor.matmul(
                            av[hp][h][:, w0:512],
                            V[:, hp, kt, 65 * h : 65 * h + 65],
                            pt[:, h, w0:512],
                            start=(kt == 0),
                            stop=(kt == nkt - 1),
                        )
                    if kt == nkt - 1:
                        attn = attnp.tile([128, 512], BF16, tag="attn", name="attn")
                        normalize(av[hp], attn)
                        attn_tiles.append(attn)
                        if qc == NSLAB - 1 and hp == 0:
                            # last chunk: hp0's projection half feeds the PE
                            # during the final softmax chain. Its steps are
                            # deliberately NOT added to total_steps, so the
                            # pacing targets never consume it and the
                            # end-of-chunk drain emits it right where the
                            # hp1 normalize would otherwise idle the PE.
                            pending.append(
                                (outproj_half(0, attn, y, qc * 512), 12)
                            )
                        # keep the PE fed while the DVE normalize chain frees
                        # the av buffers the next head-pair's AVs rotate onto
                        advance(steps_done + 10)

                while pending:
                    advance(steps_done + 100)

                if qc + 1 < NSLAB:
                    op_carry = (outproj_unit(qc, *attn_tiles), 20)
                else:
                    for _ in outproj_half(1, attn_tiles[1], y2, 0):
                        pass

    nc.compile()
    return nc


def _get_nc():
    if "nc" not in _CACHE:
        _CACHE["nc"] = _build()
    return _CACHE["nc"]


def _bf16(a):
    return np.ascontiguousarray(a.astype(ml_dtypes.bfloat16))


def _run(inputs, **spmd_kwargs):
    x = np.asarray(inputs["x"], dtype=np.float32)
    w_qkv = np.asarray(inputs["w_qkv"], dtype=np.float32)
    b_qkv = np.asarray(inputs["b_qkv"], dtype=np.float32)
    w_proj = np.asarray(inputs["w_proj"], dtype=np.float32)
    b_proj = np.asarray(inputs["b_proj"], dtype=np.float32)

    nc = _get_nc()

    in_maps = []
    for i in range(NCORES):
        b = i // NG
        g = i % NG
        f0 = g * FPC
        sl = slice(f0, f0 + FPC)
        in_maps.append(
            {
                "xT": _bf16(x[b].T),
                "wq": _bf16(w_qkv[:, sl]),
                "wk": _bf16(w_qkv[:, C + f0 : C + f0 + FPC]),
                "wv": _bf16(w_qkv[:, 2 * C + f0 : 2 * C + f0 + FPC]),
                "bq": np.ascontiguousarray(
                    b_qkv[sl].reshape(2, 128).T
                ),
                "bk": np.ascontiguousarray(
                    b_qkv[C + f0 : C + f0 + FPC].reshape(2, 128).T
                ),
                "bv": np.ascontiguousarray(
                    b_qkv[2 * C + f0 : 2 * C + f0 + FPC].reshape(2, 128).T
                ),
                "wp": _bf16(w_proj[sl, :]),
            }
        )

    res = run_bass_kernel_spmd(nc, in_maps, core_ids=list(range(NCORES)), **spmd_kwargs)
    acc = np.zeros((B, T, C), dtype=np.float64)
    for i, om in enumerate(res.results):
        acc[i // NG] += np.asarray(om["y"]).astype(np.float64)
        # last chunk's hp1 output-projection partial
        acc[i // NG, T - 512 :] += np.asarray(om["y2"]).astype(np.float64)
    out = (acc + b_proj.astype(np.float64)).astype(np.float32)
    return out, res


def kernel(**inputs) -> np.ndarray:
    out, _ = _run(inputs)
    return out
